# revision 35
# baseline (speedup 1.0000x reference)
"""Trainium2 Bass kernel for a 2-layer GRU (B=64, T=2048, I=16, H=256) + MLP regressor.

v3 strategy:
  - Data parallel: batch 64 sharded as 8 sequences per NeuronCore.
  - Both GRU layers per core, skewed by D=128 steps, each with its OWN
    per-step PSUM tile and a SHORT elementwise chain; the two layers' chains
    are emitted interleaved so layer A's matmul group and ACT ops overlap
    layer B's DVE ops (antiphase 2-stage pipeline on shared engines).
  - Per layer-step: identity-MM preloads [xr|xz|xn|0] into PSUM (starts the
    accumulation bracket; kills the xg add), 12 gate MMs accumulate, then:
      sigmoid(32->bf16) ; [GPSIMD: u=1-z, zh=z*h_prev] ;
      mul(r*hn) ; add(+xn) ; tanh(->bf16) ; mul(n*u) ; add(+zh -> h' bf16)
    The h state lives ONLY in bf16 (it is the matmul moving operand), so
    there is no cast on the serial path.
  - All matmul operands bf16 (FWL halves LDWEIGHTS, which dominates PE time
    at N=8); PSUM accumulation fp32.
  - Input-gate projections precomputed in C-step chunks into a bf16 ring;
    regressor fused every C steps.
"""

import os
import sys

import numpy as np

if "/opt/trn_rl_repo" not in sys.path:
    sys.path.insert(0, "/opt/trn_rl_repo")

import concourse.bacc as bacc
import concourse.mybir as mybir
import concourse.tile as tile
from concourse.bass import ds, ts
from concourse.bass_utils import run_bass_kernel_spmd

# Problem constants (hardcoded per harness contract)
B_TOTAL = 64
N_CORES = 8
Bc = B_TOTAL // N_CORES  # 8 sequences per core
T = 2048
I_DIM = 16
H = 256
G = 3 * H  # 768 gate rows
C = 64  # chunk size for batched precomputes
S = 128  # ring size in steps (2 chunks)
D = 128  # layer-1 skew (steps)

F32 = mybir.dt.float32
BF16 = mybir.dt.bfloat16
AF = mybir.ActivationFunctionType
ALU = mybir.AluOpType

NB = Bc              # 8: batch per core
W = 4 * NB           # 32: h-state cols per step  [L0k0|L0k1|L1k0|L1k1]
LG = 8 * NB          # 64: per-layer ring cols per step [xr|xz|xn|zeros]
GW = 2 * LG          # 128: ring cols per step [L0 | L1]

# per-layer psum layout (64 cols): [r | z | xn | hn]
PS_R, PS_Z, PS_XN, PS_HN = 0, 2 * NB, 4 * NB, 6 * NB


def _ring_col(layer, g, m):
    """xg ring per-step col offset for gate g in {'r','z','n'}, chunk m."""
    return layer * LG + {"r": 0, "z": 2 * NB, "n": 4 * NB}[g] + m * NB


def build_program(dt_compute=BF16, repeat=1):
    """Build + compile the SPMD program (identical on all 8 cores)."""
    DT = dt_compute
    nc = bacc.Bacc("TRN2", target_bir_lowering=False, debug=False,
                   num_devices=N_CORES)

    # ---- DRAM I/O ----
    xT_h = nc.dram_tensor("xT", [I_DIM + 1, T * Bc], DT, kind="ExternalInput")
    wh0_h = nc.dram_tensor("wh0T", [H, G], DT, kind="ExternalInput")
    wih0_h = nc.dram_tensor("wih0T", [I_DIM + 1, G], DT, kind="ExternalInput")
    wh1_h = nc.dram_tensor("wh1T", [H, G], DT, kind="ExternalInput")
    wih1_h = nc.dram_tensor("wih1T", [H, G], DT, kind="ExternalInput")
    ident_h = nc.dram_tensor("ident", [128, 128], DT, kind="ExternalInput")
    w1_h = nc.dram_tensor("w1T", [H, H], DT, kind="ExternalInput")
    b1_h = nc.dram_tensor("b1c", [128, 2], F32, kind="ExternalInput")
    w2_h = nc.dram_tensor("w2c", [128, 2], DT, kind="ExternalInput")
    b2_h = nc.dram_tensor("b2c", [1, 1], F32, kind="ExternalInput")
    out_h = nc.dram_tensor("out", [T // C, C * Bc], F32, kind="ExternalOutput")

    with tile.TileContext(nc) as tc:
        with (
            tc.tile_pool(name="cst", bufs=1) as cst,
            tc.tile_pool(name="work", bufs=3) as work,
            tc.tile_pool(name="pg", bufs=4, space="PSUM") as pg,
            tc.tile_pool(name="pbig", bufs=3, space="PSUM") as pbig,
            tc.tile_pool(name="pst2", bufs=1, space="PSUM") as pst2,
        ):
            # ---- persistent SBUF ----
            xT = cst.tile([I_DIM + 1, T * NB], DT, tag="xT")
            wh0 = cst.tile([128, 12 * 128], DT, tag="wh0")
            wh1 = cst.tile([128, 12 * 128], DT, tag="wh1")
            wih0 = cst.tile([I_DIM + 1, G], DT, tag="wih0")
            wih1 = cst.tile([128, 2 * G], DT, tag="wih1")
            ident = cst.tile([128, 128], DT, tag="ident")
            w1 = cst.tile([128, 4 * 128], DT, tag="w1")
            w2 = cst.tile([128, 2], DT, tag="w2")
            b1 = cst.tile([128, 2], F32, tag="b1")
            b2 = cst.tile([1, 1], F32, tag="b2")
            xg = cst.tile([128, S * GW], DT, tag="xg")      # x-gates ring
            hist = cst.tile([128, S * W], DT, tag="hist")   # bf16 h state

            # ---- load constants ----
            nc.sync.dma_start(xT[:], xT_h[:, :])
            for m in range(6):
                for k in range(2):
                    i = m * 2 + k
                    nc.sync.dma_start(wh0[:, ts(i, 128)],
                                      wh0_h[ds(k * 128, 128), ds(m * 128, 128)])
                    nc.sync.dma_start(wh1[:, ts(i, 128)],
                                      wh1_h[ds(k * 128, 128), ds(m * 128, 128)])
            nc.sync.dma_start(wih0[:], wih0_h[:, :])
            for k in range(2):
                nc.sync.dma_start(wih1[:, ts(k, G)], wih1_h[ds(k * 128, 128), :])
            nc.sync.dma_start(ident[:], ident_h[:, :])
            for mm in range(2):
                for k in range(2):
                    nc.sync.dma_start(w1[:, ts(mm * 2 + k, 128)],
                                      w1_h[ds(k * 128, 128), ds(mm * 128, 128)])
            nc.sync.dma_start(w2[:], w2_h[:, :])
            nc.sync.dma_start(b1[:], b1_h[:, :])
            nc.sync.dma_start(b2[:], b2_h[:, :])
            # One-time ring clear: establishes the permanent zeros blocks and
            # avoids NaN reads from uninitialized columns in early rounds.
            nc.vector.memset(xg[:], 0.0)

            def hist_mv(layer, t, k):
                """moving operand: h_{layer}(t) k-chunk, [128, NB] bf16."""
                slot = t % S
                return hist[:, ds(slot * W + (2 * layer + k) * NB, NB)]

            def hist_hcols(layer, t):
                """h_{layer}(t): both k-chunks, [128, 2*NB] bf16."""
                slot = t % S
                return hist[:, ds(slot * W + 2 * layer * NB, 2 * NB)]

            def emit_xg0_chunk(c):
                """layer-0 x-gates for steps [c*C, (c+1)*C); generator with
                one (matmul + ring copy) unit per advance."""
                base_step = (c * C) % S
                for m in range(6):
                    g, j = ("r", "z", "n")[m // 2], m % 2
                    ps = pbig.tile([128, C * NB], F32, tag="big")
                    nc.tensor.matmul(ps[:],
                                     wih0[:, ts(m, 128)],
                                     xT[:, ds(c * C * NB, C * NB)],
                                     start=True, stop=True)
                    dst = xg[:, ds(base_step * GW, C * GW)]
                    dst = dst.rearrange("p (s g) -> p s g", g=GW)
                    dst = dst[:, :, ds(_ring_col(0, g, j), NB)]
                    src = ps[:].rearrange("p (s b) -> p s b", b=NB)
                    nc.scalar.activation(dst, src, AF.Copy)
                    yield

            def emit_xg1_chunk(c):
                """layer-1 x-gates for steps [c*C, (c+1)*C) from the layer-0
                h history; one (2 matmuls + ring copy) unit per advance."""
                base_step = (c * C) % S
                seg = hist[:, ds(base_step * W, C * W)]
                seg = seg.rearrange("p (s c) -> p s c", c=W)
                for m in range(6):
                    g, j = ("r", "z", "n")[m // 2], m % 2
                    ps = pbig.tile([128, C * NB], F32, tag="big")
                    for k in range(2):
                        nc.tensor.matmul(ps[:],
                                         wih1[:, ds(k * G + m * 128, 128)],
                                         seg[:, :, ds(k * NB, NB)],
                                         start=(k == 0), stop=(k == 1))
                    dst = xg[:, ds(base_step * GW, C * GW)]
                    dst = dst.rearrange("p (s g) -> p s g", g=GW)
                    dst = dst[:, :, ds(_ring_col(1, g, j), NB)]
                    src = ps[:].rearrange("p (s b) -> p s b", b=NB)
                    nc.scalar.activation(dst, src, AF.Copy)
                    yield

            def emit_regressor_chunk(rc):
                """relu(h2@W1.T+b1) @ W2.T + b2 -> relu -> out for steps
                [rc*C, (rc+1)*C) of layer 1; 3 units."""
                base_step = (rc * C) % S
                seg = hist[:, ds(base_step * W, C * W)]
                seg = seg.rearrange("p (s c) -> p s c", c=W)
                rT = work.tile([128, 2 * C * NB], DT, tag="rT")
                for mm in range(2):
                    ps = pbig.tile([128, C * NB], F32, tag="big")
                    for k in range(2):
                        nc.tensor.matmul(ps[:],
                                         w1[:, ts(mm * 2 + k, 128)],
                                         seg[:, :, ds((2 + k) * NB, NB)],
                                         start=(k == 0), stop=(k == 1))
                    nc.scalar.activation(rT[:, ts(mm, C * NB)], ps[:],
                                         AF.Relu, bias=b1[:, ds(mm, 1)])
                    yield
                po = pst2.tile([1, C * NB], F32, tag="st2")
                for k in range(2):
                    nc.tensor.matmul(po[:], w2[:, ds(k, 1)],
                                     rT[:, ts(k, C * NB)],
                                     start=(k == 0), stop=(k == 1))
                oT = work.tile([1, C * NB], F32, tag="oT")
                nc.scalar.activation(oT[:], po[:], AF.Relu, bias=b2[:, ds(0, 1)])
                nc.sync.dma_start(out_h[ds(rc, 1), :], oT[:])
                yield

            def emit_mm_group(layer, t):
                """identity preload + 12 gate MMs for one layer-step; returns
                the psum tile ([r | z | xn | hn], one bank)."""
                slot = t % S
                wh = wh0 if layer == 0 else wh1
                ps = pg.tile([128, 8 * NB], F32, tag="ps")
                nc.tensor.matmul(ps[:],
                                 ident[:, :],
                                 xg[:, ds(slot * GW + layer * LG, LG)],
                                 start=True, stop=False)
                mms = [(g, m, k)
                       for g in ("r", "z", "n") for m in range(2)
                       for k in range(2)]
                for i, (g, m, k) in enumerate(mms):
                    mrow = {"r": 0, "z": 2, "n": 4}[g] + m
                    pcol = {"r": PS_R, "z": PS_Z, "n": PS_HN}[g] + m * NB
                    nc.tensor.matmul(
                        ps[:, ds(pcol, NB)],
                        wh[:, ts(mrow * 2 + k, 128)],
                        hist_mv(layer, t - 1, k),
                        start=False, stop=(i == len(mms) - 1))
                return ps

            def chain_gen(layer, t, ps):
                """Short per-layer elementwise chain; yields between ops so
                two layers' chains interleave in emission order."""
                rz = work.tile([128, 4 * NB], BF16, tag=f"rz{layer}")
                nc.scalar.activation(rz[:], ps[:, 0:4 * NB], AF.Sigmoid)
                yield
                # z-path on GPSIMD (off the serial path, runs during tanh)
                u = work.tile([128, 2 * NB], BF16, tag=f"u{layer}")
                nc.gpsimd.tensor_scalar(u[:], rz[:, ds(2 * NB, 2 * NB)],
                                        -1.0, 1.0, ALU.mult, ALU.add)
                zh = work.tile([128, 2 * NB], F32, tag=f"zh{layer}")
                nc.gpsimd.tensor_mul(zh[:], rz[:, ds(2 * NB, 2 * NB)],
                                     hist_hcols(layer, t - 1))
                yield
                tt = work.tile([128, 2 * NB], F32, tag=f"tt{layer}")
                nc.vector.tensor_mul(tt[:], rz[:, ds(0, 2 * NB)],
                                     ps[:, ds(PS_HN, 2 * NB)])
                yield
                t2 = work.tile([128, 2 * NB], F32, tag=f"t2{layer}")
                nc.vector.tensor_add(t2[:], tt[:], ps[:, ds(PS_XN, 2 * NB)])
                yield
                nn = work.tile([128, 2 * NB], BF16, tag=f"nn{layer}")
                nc.scalar.activation(nn[:], t2[:], AF.Tanh)
                yield
                nu = work.tile([128, 2 * NB], F32, tag=f"nu{layer}")
                if os.environ.get("KTAILGPS", "0") == "1":
                    nc.gpsimd.tensor_mul(nu[:], nn[:], u[:])
                    nc.gpsimd.tensor_add(hist_hcols(layer, t), nu[:], zh[:])
                else:
                    nc.vector.tensor_mul(nu[:], nn[:], u[:])
                    yield
                    nc.vector.tensor_add(hist_hcols(layer, t), nu[:], zh[:])

            def emit_round(r):
                work_items = []
                if r < T:
                    work_items.append((0, r))
                if r >= D:
                    work_items.append((1, r - D))
                gens = []
                for layer, t in work_items:
                    ps = emit_mm_group(layer, t)
                    gens.append(chain_gen(layer, t, ps))
                while gens:
                    gens = [g for g in gens if next(g, "done") != "done"]

            no_aux = os.environ.get("KNOAUX", "0") == "1"
            bulk_aux = os.environ.get("KSPREAD", "0") != "1"

            def drain(gen):
                for _ in gen:
                    pass

            def emit_body():
                # zero initial h slots (slot S-1 == slot(-1))
                nc.vector.memset(hist[:, ds((S - 1) * W, W)], 0.0)
                aux = []  # active aux generators, advanced 1 unit per round
                if no_aux:
                    nc.vector.memset(xg[:], 0.01)
                else:
                    drain(emit_xg0_chunk(0))
                    drain(emit_xg0_chunk(1))
                n_rounds = T + D
                for r in range(n_rounds):
                    # advance the aux queue by one unit per round, emitted
                    # ahead of the round's chain ops so the copies fill the
                    # engines' idle window at round start
                    if aux and (bulk_aux or next(aux[0], "done") == "done"):
                        if bulk_aux:
                            for g in aux:
                                drain(g)
                            aux = []
                        else:
                            aux.pop(0)
                    emit_round(r)
                    if no_aux:
                        continue
                    if r < T and (r + 1) % C == 0:
                        c = (r + 1) // C - 1  # layer-0 chunk just finished
                        if c + 2 < T // C:
                            aux.append(emit_xg0_chunk(c + 2))
                        aux.append(emit_xg1_chunk(c))
                    if r >= D and (r - D + 1) % C == 0:
                        aux.append(emit_regressor_chunk((r - D + 1) // C - 1))
                for g in aux:
                    drain(g)
                if no_aux:
                    drain(emit_regressor_chunk(0))

            if repeat == 1:
                emit_body()
            else:
                with tc.For_i(0, repeat, 1):
                    emit_body()

    nc.compile()
    return nc


_CACHE = {}


def _get_program(dt=BF16, repeat=1):
    key = (str(dt), repeat)
    if key not in _CACHE:
        _CACHE[key] = build_program(dt, repeat)
    return _CACHE[key]


def make_in_maps(inputs, np_dt=None):
    """Host-side prep: slice batch, transpose, pack biases, cast bf16."""
    import ml_dtypes
    if np_dt is None:
        np_dt = ml_dtypes.bfloat16
    x = np.asarray(inputs["x"], np.float32)
    Wih0 = np.asarray(inputs["Wih0"], np.float32)
    Whh0 = np.asarray(inputs["Whh0"], np.float32)
    bih0 = np.asarray(inputs["bih0"], np.float32)
    bhh0 = np.asarray(inputs["bhh0"], np.float32)
    Wih1 = np.asarray(inputs["Wih1"], np.float32)
    Whh1 = np.asarray(inputs["Whh1"], np.float32)
    bih1 = np.asarray(inputs["bih1"], np.float32)
    bhh1 = np.asarray(inputs["bhh1"], np.float32)
    W1 = np.asarray(inputs["W1"], np.float32)
    b1 = np.asarray(inputs["b1"], np.float32)
    W2 = np.asarray(inputs["W2"], np.float32)
    b2 = np.asarray(inputs["b2"], np.float32)

    assert not np.any(bhh0[2 * H:]) and not np.any(bhh1[2 * H:]), \
        "nonzero bhh n-gate bias not supported by this build"
    assert not np.any(bih1) and not np.any(bhh1[:2 * H]), \
        "nonzero layer-1 input bias not supported by this build"

    bias0 = np.concatenate([bih0[:2 * H] + bhh0[:2 * H], bih0[2 * H:]])
    wih0T = np.vstack([Wih0.T, bias0[None, :]]).astype(np_dt)  # [17, 768]

    shared = {
        "wh0T": Whh0.T.copy().astype(np_dt),
        "wih0T": wih0T,
        "wh1T": Whh1.T.copy().astype(np_dt),
        "wih1T": Wih1.T.copy().astype(np_dt),
        "ident": np.eye(128, dtype=np_dt),
        "w1T": W1.T.copy().astype(np_dt),
        "b1c": b1.reshape(2, 128).T.copy().astype(np.float32),
        "w2c": W2[0].reshape(2, 128).T.copy().astype(np_dt),
        "b2c": b2.reshape(1, 1).astype(np.float32),
    }
    in_maps = []
    for c in range(N_CORES):
        xc = x[c * Bc:(c + 1) * Bc]  # [8, T, 16]
        xTc = xc.transpose(2, 1, 0).reshape(I_DIM, T * Bc)  # [16, T*8]
        xTc = np.vstack([xTc, np.ones((1, T * Bc), np.float32)]).astype(np_dt)
        m = dict(shared)
        m["xT"] = xTc
        in_maps.append(m)
    return in_maps


def assemble_output(results):
    outs = []
    for c in range(N_CORES):
        r = np.asarray(results[c]["out"], np.float32)  # [32, 512]
        r = r.reshape(T // C, C, Bc).transpose(2, 0, 1).reshape(Bc, T)
        outs.append(r)
    return np.concatenate(outs, axis=0)[:, :, None]  # [64, 2048, 1]


DT_COMPUTE = BF16
NP_DT = None  # resolved to ml_dtypes.bfloat16 in make_in_maps


def kernel(**inputs):
    nc = _get_program(DT_COMPUTE, 1)
    in_maps = make_in_maps(inputs)
    res = run_bass_kernel_spmd(nc, in_maps, core_ids=list(range(N_CORES)))
    return assemble_output(res.results)


# revision 40
# speedup vs baseline: 1.0522x; 1.0522x over previous
"""Trainium2 Bass kernel for a 2-layer GRU (B=64, T=2048, I=16, H=256) + MLP regressor.

v3 strategy:
  - Data parallel: batch 64 sharded as 8 sequences per NeuronCore.
  - Both GRU layers per core, skewed by D=128 steps, each with its OWN
    per-step PSUM tile and a SHORT elementwise chain; the two layers' chains
    are emitted interleaved so layer A's matmul group and ACT ops overlap
    layer B's DVE ops (antiphase 2-stage pipeline on shared engines).
  - Per layer-step: identity-MM preloads [xr|xz|xn|0] into PSUM (starts the
    accumulation bracket; kills the xg add), 12 gate MMs accumulate, then:
      sigmoid(32->bf16) ; [GPSIMD: u=1-z, zh=z*h_prev] ;
      mul(r*hn) ; add(+xn) ; tanh(->bf16) ; mul(n*u) ; add(+zh -> h' bf16)
    The h state lives ONLY in bf16 (it is the matmul moving operand), so
    there is no cast on the serial path.
  - All matmul operands bf16 (FWL halves LDWEIGHTS, which dominates PE time
    at N=8); PSUM accumulation fp32.
  - Input-gate projections precomputed in C-step chunks into a bf16 ring;
    regressor fused every C steps.
"""

import os
import sys

import numpy as np

if "/opt/trn_rl_repo" not in sys.path:
    sys.path.insert(0, "/opt/trn_rl_repo")

import concourse.bacc as bacc
import concourse.mybir as mybir
import concourse.tile as tile
from concourse.bass import ds, ts
from concourse.bass_utils import run_bass_kernel_spmd

# Problem constants (hardcoded per harness contract)
B_TOTAL = 64
N_CORES = 8
Bc = B_TOTAL // N_CORES  # 8 sequences per core
T = 2048
I_DIM = 16
H = 256
G = 3 * H  # 768 gate rows
C = 64  # chunk size for batched precomputes
S = 128  # ring size in steps (2 chunks)
D = 128  # layer-1 skew (steps)

F32 = mybir.dt.float32
BF16 = mybir.dt.bfloat16
AF = mybir.ActivationFunctionType
ALU = mybir.AluOpType

NB = Bc              # 8: batch per core
W = 4 * NB           # 32: h-state cols per step  [L0k0|L0k1|L1k0|L1k1]
LG = 8 * NB          # 64: per-layer ring cols per step [xr|xz|xn|zeros]
GW = 2 * LG          # 128: ring cols per step [L0 | L1]

# per-layer psum layout (64 cols): [r | z | xn | hn]
PS_R, PS_Z, PS_XN, PS_HN = 0, 2 * NB, 4 * NB, 6 * NB


def _ring_col(layer, g, m):
    """xg ring per-step col offset for gate g in {'r','z','n'}, chunk m."""
    return layer * LG + {"r": 0, "z": 2 * NB, "n": 4 * NB}[g] + m * NB


def build_program(dt_compute=BF16, repeat=1):
    """Build + compile the SPMD program (identical on all 8 cores)."""
    DT = dt_compute
    nc = bacc.Bacc("TRN2", target_bir_lowering=False, debug=False,
                   num_devices=N_CORES)

    # ---- DRAM I/O ----
    xT_h = nc.dram_tensor("xT", [I_DIM + 1, T * Bc], DT, kind="ExternalInput")
    wh0_h = nc.dram_tensor("wh0T", [H, G], DT, kind="ExternalInput")
    wih0_h = nc.dram_tensor("wih0T", [I_DIM + 1, G], DT, kind="ExternalInput")
    wh1_h = nc.dram_tensor("wh1T", [H, G], DT, kind="ExternalInput")
    wih1_h = nc.dram_tensor("wih1T", [H, G], DT, kind="ExternalInput")
    ident_h = nc.dram_tensor("ident", [128, 128], DT, kind="ExternalInput")
    w1_h = nc.dram_tensor("w1T", [H, H], DT, kind="ExternalInput")
    b1_h = nc.dram_tensor("b1c", [128, 2], F32, kind="ExternalInput")
    w2_h = nc.dram_tensor("w2c", [128, 2], DT, kind="ExternalInput")
    b2_h = nc.dram_tensor("b2c", [1, 1], F32, kind="ExternalInput")
    out_h = nc.dram_tensor("out", [T // C, C * Bc], F32, kind="ExternalOutput")

    with tile.TileContext(nc) as tc:
        with (
            tc.tile_pool(name="cst", bufs=1) as cst,
            tc.tile_pool(name="work", bufs=3) as work,
            tc.tile_pool(name="pgA", bufs=3, space="PSUM") as pgA,
            tc.tile_pool(name="pgB", bufs=2, space="PSUM") as pgB,
            tc.tile_pool(name="pbig", bufs=2, space="PSUM") as pbig,
            tc.tile_pool(name="pst2", bufs=1, space="PSUM") as pst2,
        ):
            # ---- persistent SBUF ----
            xT = cst.tile([I_DIM + 1, T * NB], DT, tag="xT")
            wh0 = cst.tile([128, 12 * 128], DT, tag="wh0")
            wh1 = cst.tile([128, 12 * 128], DT, tag="wh1")
            wih0 = cst.tile([I_DIM + 1, G], DT, tag="wih0")
            wih1 = cst.tile([128, 2 * G], DT, tag="wih1")
            ident = cst.tile([128, 128], DT, tag="ident")
            w1 = cst.tile([128, 4 * 128], DT, tag="w1")
            w2 = cst.tile([128, 2], DT, tag="w2")
            b1 = cst.tile([128, 2], F32, tag="b1")
            b2 = cst.tile([1, 1], F32, tag="b2")
            xg = cst.tile([128, S * GW], DT, tag="xg")      # x-gates ring
            hist = cst.tile([128, S * W], DT, tag="hist")   # bf16 h state

            # ---- load constants ----
            nc.sync.dma_start(xT[:], xT_h[:, :])
            for m in range(6):
                for k in range(2):
                    i = m * 2 + k
                    nc.sync.dma_start(wh0[:, ts(i, 128)],
                                      wh0_h[ds(k * 128, 128), ds(m * 128, 128)])
                    nc.sync.dma_start(wh1[:, ts(i, 128)],
                                      wh1_h[ds(k * 128, 128), ds(m * 128, 128)])
            nc.sync.dma_start(wih0[:], wih0_h[:, :])
            for k in range(2):
                nc.sync.dma_start(wih1[:, ts(k, G)], wih1_h[ds(k * 128, 128), :])
            nc.sync.dma_start(ident[:], ident_h[:, :])
            for mm in range(2):
                for k in range(2):
                    nc.sync.dma_start(w1[:, ts(mm * 2 + k, 128)],
                                      w1_h[ds(k * 128, 128), ds(mm * 128, 128)])
            nc.sync.dma_start(w2[:], w2_h[:, :])
            nc.sync.dma_start(b1[:], b1_h[:, :])
            nc.sync.dma_start(b2[:], b2_h[:, :])
            # One-time ring clear: establishes the permanent zeros blocks and
            # avoids NaN reads from uninitialized columns in early rounds.
            nc.vector.memset(xg[:], 0.0)

            def hist_mv(layer, t, k):
                """moving operand: h_{layer}(t) k-chunk, [128, NB] bf16."""
                slot = t % S
                return hist[:, ds(slot * W + (2 * layer + k) * NB, NB)]

            def hist_hcols(layer, t):
                """h_{layer}(t): both k-chunks, [128, 2*NB] bf16."""
                slot = t % S
                return hist[:, ds(slot * W + 2 * layer * NB, 2 * NB)]

            def emit_xg0_chunk(c):
                """layer-0 x-gates for steps [c*C, (c+1)*C); generator with
                one (matmul + ring copy) unit per advance."""
                base_step = (c * C) % S
                for m in range(6):
                    g, j = ("r", "z", "n")[m // 2], m % 2
                    ps = pbig.tile([128, C * NB], F32, tag="big")
                    nc.tensor.matmul(ps[:],
                                     wih0[:, ts(m, 128)],
                                     xT[:, ds(c * C * NB, C * NB)],
                                     start=True, stop=True)
                    dst = xg[:, ds(base_step * GW, C * GW)]
                    dst = dst.rearrange("p (s g) -> p s g", g=GW)
                    dst = dst[:, :, ds(_ring_col(0, g, j), NB)]
                    src = ps[:].rearrange("p (s b) -> p s b", b=NB)
                    nc.scalar.activation(dst, src, AF.Copy)
                    yield

            def emit_xg1_chunk(c):
                """layer-1 x-gates for steps [c*C, (c+1)*C) from the layer-0
                h history; one (2 matmuls + ring copy) unit per advance."""
                base_step = (c * C) % S
                seg = hist[:, ds(base_step * W, C * W)]
                seg = seg.rearrange("p (s c) -> p s c", c=W)
                for m in range(6):
                    g, j = ("r", "z", "n")[m // 2], m % 2
                    ps = pbig.tile([128, C * NB], F32, tag="big")
                    for k in range(2):
                        nc.tensor.matmul(ps[:],
                                         wih1[:, ds(k * G + m * 128, 128)],
                                         seg[:, :, ds(k * NB, NB)],
                                         start=(k == 0), stop=(k == 1))
                    dst = xg[:, ds(base_step * GW, C * GW)]
                    dst = dst.rearrange("p (s g) -> p s g", g=GW)
                    dst = dst[:, :, ds(_ring_col(1, g, j), NB)]
                    src = ps[:].rearrange("p (s b) -> p s b", b=NB)
                    nc.scalar.activation(dst, src, AF.Copy)
                    yield

            def emit_regressor_chunk(rc):
                """relu(h2@W1.T+b1) @ W2.T + b2 -> relu -> out for steps
                [rc*C, (rc+1)*C) of layer 1; 3 units."""
                base_step = (rc * C) % S
                seg = hist[:, ds(base_step * W, C * W)]
                seg = seg.rearrange("p (s c) -> p s c", c=W)
                rT = work.tile([128, 2 * C * NB], DT, tag="rT")
                for mm in range(2):
                    ps = pbig.tile([128, C * NB], F32, tag="big")
                    for k in range(2):
                        nc.tensor.matmul(ps[:],
                                         w1[:, ts(mm * 2 + k, 128)],
                                         seg[:, :, ds((2 + k) * NB, NB)],
                                         start=(k == 0), stop=(k == 1))
                    nc.scalar.activation(rT[:, ts(mm, C * NB)], ps[:],
                                         AF.Relu, bias=b1[:, ds(mm, 1)])
                    yield
                po = pst2.tile([1, C * NB], F32, tag="st2")
                for k in range(2):
                    nc.tensor.matmul(po[:], w2[:, ds(k, 1)],
                                     rT[:, ts(k, C * NB)],
                                     start=(k == 0), stop=(k == 1))
                oT = work.tile([1, C * NB], F32, tag="oT")
                nc.scalar.activation(oT[:], po[:], AF.Relu, bias=b2[:, ds(0, 1)])
                nc.sync.dma_start(out_h[ds(rc, 1), :], oT[:])
                yield

            def emit_mm_group(layer, t):
                """Gate matmuls for one layer-step, two PSUM banks:
                  A [r|z|xn]: identity preload of xr|xz|xn + 8 rz MMs -> the
                    sigmoid unblocks after 9 MMs instead of 13.
                  B [hn]: 4 n MMs; the first carries start=True (a start marks
                    the whole bank pending-zero, so no preload is needed).
                Returns (psA, psB)."""
                slot = t % S
                wh = wh0 if layer == 0 else wh1
                psA = pgA.tile([128, 6 * NB], F32, tag="psA")
                psB = pgB.tile([128, 2 * NB], F32, tag="psB")
                nc.tensor.matmul(psA[:],
                                 ident[:, :],
                                 xg[:, ds(slot * GW + layer * LG, 6 * NB)],
                                 start=True, stop=False)
                mms = [(g, m, k)
                       for g in ("r", "z") for m in range(2)
                       for k in range(2)]
                for i, (g, m, k) in enumerate(mms):
                    mrow = {"r": 0, "z": 2}[g] + m
                    pcol = {"r": PS_R, "z": PS_Z}[g] + m * NB
                    nc.tensor.matmul(
                        psA[:, ds(pcol, NB)],
                        wh[:, ts(mrow * 2 + k, 128)],
                        hist_mv(layer, t - 1, k),
                        start=False, stop=(i == len(mms) - 1))
                nmm = [(m, k) for m in range(2) for k in range(2)]
                for i, (m, k) in enumerate(nmm):
                    nc.tensor.matmul(
                        psB[:, ds(m * NB, NB)],
                        wh[:, ts((4 + m) * 2 + k, 128)],
                        hist_mv(layer, t - 1, k),
                        start=(i == 0), stop=(i == len(nmm) - 1))
                return psA, psB

            def chain_gen(layer, t, psA, psB):
                """Short per-layer elementwise chain; yields between ops so
                two layers' chains interleave in emission order."""
                rz = work.tile([128, 4 * NB], BF16, tag=f"rz{layer}")
                nc.scalar.activation(rz[:], psA[:, 0:4 * NB], AF.Sigmoid)
                yield
                # z-path on GPSIMD (off the serial path, runs during tanh)
                u = work.tile([128, 2 * NB], BF16, tag=f"u{layer}")
                nc.gpsimd.tensor_scalar(u[:], rz[:, ds(2 * NB, 2 * NB)],
                                        -1.0, 1.0, ALU.mult, ALU.add)
                zh = work.tile([128, 2 * NB], F32, tag=f"zh{layer}")
                nc.gpsimd.tensor_mul(zh[:], rz[:, ds(2 * NB, 2 * NB)],
                                     hist_hcols(layer, t - 1))
                yield
                tt = work.tile([128, 2 * NB], F32, tag=f"tt{layer}")
                nc.vector.tensor_mul(tt[:], rz[:, ds(0, 2 * NB)], psB[:])
                yield
                t2 = work.tile([128, 2 * NB], F32, tag=f"t2{layer}")
                nc.vector.tensor_add(t2[:], tt[:], psA[:, ds(PS_XN, 2 * NB)])
                yield
                nn = work.tile([128, 2 * NB], BF16, tag=f"nn{layer}")
                nc.scalar.activation(nn[:], t2[:], AF.Tanh)
                yield
                nu = work.tile([128, 2 * NB], F32, tag=f"nu{layer}")
                if os.environ.get("KTAILGPS", "0") == "1":
                    nc.gpsimd.tensor_mul(nu[:], nn[:], u[:])
                    nc.gpsimd.tensor_add(hist_hcols(layer, t), nu[:], zh[:])
                else:
                    nc.vector.tensor_mul(nu[:], nn[:], u[:])
                    yield
                    nc.vector.tensor_add(hist_hcols(layer, t), nu[:], zh[:])

            def emit_round(r):
                work_items = []
                if r < T:
                    work_items.append((0, r))
                if r >= D:
                    work_items.append((1, r - D))
                gens = []
                for layer, t in work_items:
                    psA, psB = emit_mm_group(layer, t)
                    gens.append(chain_gen(layer, t, psA, psB))
                while gens:
                    gens = [g for g in gens if next(g, "done") != "done"]

            no_aux = os.environ.get("KNOAUX", "0") == "1"
            bulk_aux = os.environ.get("KSPREAD", "0") != "1"

            def drain(gen):
                for _ in gen:
                    pass

            def emit_body():
                # zero initial h slots (slot S-1 == slot(-1))
                nc.vector.memset(hist[:, ds((S - 1) * W, W)], 0.0)
                aux = []  # active aux generators, advanced 1 unit per round
                if no_aux:
                    nc.vector.memset(xg[:], 0.01)
                else:
                    drain(emit_xg0_chunk(0))
                    drain(emit_xg0_chunk(1))
                n_rounds = T + D
                for r in range(n_rounds):
                    # advance the aux queue by one unit per round, emitted
                    # ahead of the round's chain ops so the copies fill the
                    # engines' idle window at round start
                    if aux and (bulk_aux or next(aux[0], "done") == "done"):
                        if bulk_aux:
                            for g in aux:
                                drain(g)
                            aux = []
                        else:
                            aux.pop(0)
                    emit_round(r)
                    if no_aux:
                        continue
                    if r < T and (r + 1) % C == 0:
                        c = (r + 1) // C - 1  # layer-0 chunk just finished
                        if c + 2 < T // C:
                            aux.append(emit_xg0_chunk(c + 2))
                        aux.append(emit_xg1_chunk(c))
                    if r >= D and (r - D + 1) % C == 0:
                        aux.append(emit_regressor_chunk((r - D + 1) // C - 1))
                for g in aux:
                    drain(g)
                if no_aux:
                    drain(emit_regressor_chunk(0))

            if repeat == 1:
                emit_body()
            else:
                with tc.For_i(0, repeat, 1):
                    emit_body()

    nc.compile()
    return nc


_CACHE = {}


def _get_program(dt=BF16, repeat=1):
    key = (str(dt), repeat)
    if key not in _CACHE:
        _CACHE[key] = build_program(dt, repeat)
    return _CACHE[key]


def make_in_maps(inputs, np_dt=None):
    """Host-side prep: slice batch, transpose, pack biases, cast bf16."""
    import ml_dtypes
    if np_dt is None:
        np_dt = ml_dtypes.bfloat16
    x = np.asarray(inputs["x"], np.float32)
    Wih0 = np.asarray(inputs["Wih0"], np.float32)
    Whh0 = np.asarray(inputs["Whh0"], np.float32)
    bih0 = np.asarray(inputs["bih0"], np.float32)
    bhh0 = np.asarray(inputs["bhh0"], np.float32)
    Wih1 = np.asarray(inputs["Wih1"], np.float32)
    Whh1 = np.asarray(inputs["Whh1"], np.float32)
    bih1 = np.asarray(inputs["bih1"], np.float32)
    bhh1 = np.asarray(inputs["bhh1"], np.float32)
    W1 = np.asarray(inputs["W1"], np.float32)
    b1 = np.asarray(inputs["b1"], np.float32)
    W2 = np.asarray(inputs["W2"], np.float32)
    b2 = np.asarray(inputs["b2"], np.float32)

    assert not np.any(bhh0[2 * H:]) and not np.any(bhh1[2 * H:]), \
        "nonzero bhh n-gate bias not supported by this build"
    assert not np.any(bih1) and not np.any(bhh1[:2 * H]), \
        "nonzero layer-1 input bias not supported by this build"

    bias0 = np.concatenate([bih0[:2 * H] + bhh0[:2 * H], bih0[2 * H:]])
    wih0T = np.vstack([Wih0.T, bias0[None, :]]).astype(np_dt)  # [17, 768]

    shared = {
        "wh0T": Whh0.T.copy().astype(np_dt),
        "wih0T": wih0T,
        "wh1T": Whh1.T.copy().astype(np_dt),
        "wih1T": Wih1.T.copy().astype(np_dt),
        "ident": np.eye(128, dtype=np_dt),
        "w1T": W1.T.copy().astype(np_dt),
        "b1c": b1.reshape(2, 128).T.copy().astype(np.float32),
        "w2c": W2[0].reshape(2, 128).T.copy().astype(np_dt),
        "b2c": b2.reshape(1, 1).astype(np.float32),
    }
    in_maps = []
    for c in range(N_CORES):
        xc = x[c * Bc:(c + 1) * Bc]  # [8, T, 16]
        xTc = xc.transpose(2, 1, 0).reshape(I_DIM, T * Bc)  # [16, T*8]
        xTc = np.vstack([xTc, np.ones((1, T * Bc), np.float32)]).astype(np_dt)
        m = dict(shared)
        m["xT"] = xTc
        in_maps.append(m)
    return in_maps


def assemble_output(results):
    outs = []
    for c in range(N_CORES):
        r = np.asarray(results[c]["out"], np.float32)  # [32, 512]
        r = r.reshape(T // C, C, Bc).transpose(2, 0, 1).reshape(Bc, T)
        outs.append(r)
    return np.concatenate(outs, axis=0)[:, :, None]  # [64, 2048, 1]


DT_COMPUTE = BF16
NP_DT = None  # resolved to ml_dtypes.bfloat16 in make_in_maps


def kernel(**inputs):
    nc = _get_program(DT_COMPUTE, 1)
    in_maps = make_in_maps(inputs)
    res = run_bass_kernel_spmd(nc, in_maps, core_ids=list(range(N_CORES)))
    return assemble_output(res.results)


# revision 43
# speedup vs baseline: 1.2046x; 1.1449x over previous
"""Trainium2 Bass kernel for a 2-layer GRU (B=64, T=2048, I=16, H=256) + MLP regressor.

Strategy:
  - Data parallel: batch 64 sharded as 8 sequences per NeuronCore.
  - Both GRU layers per core, skewed by D=128 steps, each with its own
    per-step PSUM tiles and a SHORT elementwise chain; the two layers' chains
    are emitted interleaved so layer A's matmul group and ACT ops overlap
    layer B's DVE ops (antiphase 2-stage pipeline on shared engines).
  - Per layer-step, gate matmuls in TWO psum banks so the sigmoid's bank
    dependency covers only 9 MMs:
      bank A [r|z|xn]: identity-MM preloads xr|xz|xn (starts the bracket and
        kills the xg add) + 8 rz MMs;
      bank B [hn]: 4 n MMs, self-started (start=True marks the whole bank
        pending-zero, so no preload is needed).
    Then: sigmoid(32->bf16) ; [GPSIMD: u=1-z, zh=z*h_prev] ;
      mul(r*hn) ; add(+xn) ; tanh(->bf16) ; mul(n*u) ; add(+zh -> h' bf16)
    The h state lives ONLY in bf16 (it is the matmul moving operand), so
    there is no cast on the serial path.
  - All matmul operands bf16 (FWL halves LDWEIGHTS, which dominates PE time
    at N=8); PSUM accumulation fp32.
  - Input-gate projections precomputed in C-step chunks into a bf16 ring;
    regressor fused every C steps.
"""

import os
import sys

import numpy as np

if "/opt/trn_rl_repo" not in sys.path:
    sys.path.insert(0, "/opt/trn_rl_repo")

import concourse.bacc as bacc
import concourse.mybir as mybir
import concourse.tile as tile
from concourse.bass import ds, ts
from concourse.bass_utils import run_bass_kernel_spmd

# Problem constants (hardcoded per harness contract)
B_TOTAL = 64
N_CORES = 8
Bc = B_TOTAL // N_CORES  # 8 sequences per core
T = 2048
I_DIM = 16
H = 256
G = 3 * H  # 768 gate rows
C = 64  # chunk size for batched precomputes
S = 128  # ring size in steps (2 chunks)
D = 128  # layer-1 skew (steps)

F32 = mybir.dt.float32
BF16 = mybir.dt.bfloat16
AF = mybir.ActivationFunctionType
ALU = mybir.AluOpType

NB = Bc              # 8: batch per core
W = 4 * NB           # 32: h-state cols per step  [L0k0|L0k1|L1k0|L1k1]
LG = 8 * NB          # 64: per-layer ring cols per step [xr|xz|xn|zeros]
GW = 2 * LG          # 128: ring cols per step [L0 | L1]

# per-layer psum layout (64 cols): [r | z | xn | hn]
PS_R, PS_Z, PS_XN, PS_HN = 0, 2 * NB, 4 * NB, 6 * NB


def _ring_col(layer, g, m):
    """xg ring per-step col offset for gate g in {'r','z','n'}, chunk m."""
    return layer * LG + {"r": 0, "z": 2 * NB, "n": 4 * NB}[g] + m * NB


def build_program(dt_compute=BF16, repeat=1):
    """Build + compile the SPMD program (identical on all 8 cores)."""
    DT = dt_compute
    nc = bacc.Bacc("TRN2", target_bir_lowering=False, debug=False,
                   num_devices=N_CORES)

    # ---- DRAM I/O ----
    xT_h = nc.dram_tensor("xT", [I_DIM + 1, T * Bc], DT, kind="ExternalInput")
    wh0_h = nc.dram_tensor("wh0T", [H, G], DT, kind="ExternalInput")
    wih0_h = nc.dram_tensor("wih0T", [I_DIM + 1, G], DT, kind="ExternalInput")
    wh1_h = nc.dram_tensor("wh1T", [H, G], DT, kind="ExternalInput")
    wih1_h = nc.dram_tensor("wih1T", [H, G], DT, kind="ExternalInput")
    ident_h = nc.dram_tensor("ident", [128, 128], DT, kind="ExternalInput")
    w1_h = nc.dram_tensor("w1T", [H, H], DT, kind="ExternalInput")
    b1_h = nc.dram_tensor("b1c", [128, 2], F32, kind="ExternalInput")
    w2_h = nc.dram_tensor("w2c", [128, 2], DT, kind="ExternalInput")
    b2_h = nc.dram_tensor("b2c", [1, 1], F32, kind="ExternalInput")
    out_h = nc.dram_tensor("out", [T // C, C * Bc], F32, kind="ExternalOutput")

    with tile.TileContext(nc) as tc:
        with (
            tc.tile_pool(name="cst", bufs=1) as cst,
            tc.tile_pool(name="work", bufs=3) as work,
            tc.tile_pool(name="pgA", bufs=3, space="PSUM") as pgA,
            tc.tile_pool(name="pgB", bufs=2, space="PSUM") as pgB,
            tc.tile_pool(name="pbig", bufs=2, space="PSUM") as pbig,
            tc.tile_pool(name="pst2", bufs=1, space="PSUM") as pst2,
        ):
            # ---- persistent SBUF ----
            xT = cst.tile([I_DIM + 1, T * NB], DT, tag="xT")
            wh0 = cst.tile([128, 12 * 128], DT, tag="wh0")
            wh1 = cst.tile([128, 12 * 128], DT, tag="wh1")
            wih0 = cst.tile([I_DIM + 1, G], DT, tag="wih0")
            wih1 = cst.tile([128, 2 * G], DT, tag="wih1")
            ident = cst.tile([128, 128], DT, tag="ident")
            w1 = cst.tile([128, 4 * 128], DT, tag="w1")
            w2 = cst.tile([128, 2], DT, tag="w2")
            b1 = cst.tile([128, 2], F32, tag="b1")
            b2 = cst.tile([1, 1], F32, tag="b2")
            xg = cst.tile([128, S * GW], DT, tag="xg")      # x-gates ring
            hist = cst.tile([128, S * W], DT, tag="hist")   # bf16 h state

            # ---- load constants ----
            nc.sync.dma_start(xT[:], xT_h[:, :])
            for m in range(6):
                for k in range(2):
                    i = m * 2 + k
                    nc.sync.dma_start(wh0[:, ts(i, 128)],
                                      wh0_h[ds(k * 128, 128), ds(m * 128, 128)])
                    nc.sync.dma_start(wh1[:, ts(i, 128)],
                                      wh1_h[ds(k * 128, 128), ds(m * 128, 128)])
            nc.sync.dma_start(wih0[:], wih0_h[:, :])
            for k in range(2):
                nc.sync.dma_start(wih1[:, ts(k, G)], wih1_h[ds(k * 128, 128), :])
            nc.sync.dma_start(ident[:], ident_h[:, :])
            for mm in range(2):
                for k in range(2):
                    nc.sync.dma_start(w1[:, ts(mm * 2 + k, 128)],
                                      w1_h[ds(k * 128, 128), ds(mm * 128, 128)])
            nc.sync.dma_start(w2[:], w2_h[:, :])
            nc.sync.dma_start(b1[:], b1_h[:, :])
            nc.sync.dma_start(b2[:], b2_h[:, :])
            # One-time ring clear: establishes the permanent zeros blocks and
            # avoids NaN reads from uninitialized columns in early rounds.
            nc.vector.memset(xg[:], 0.0)

            def hist_mv(layer, t, k):
                """moving operand: h_{layer}(t) k-chunk, [128, NB] bf16."""
                slot = t % S
                return hist[:, ds(slot * W + (2 * layer + k) * NB, NB)]

            def hist_hcols(layer, t):
                """h_{layer}(t): both k-chunks, [128, 2*NB] bf16."""
                slot = t % S
                return hist[:, ds(slot * W + 2 * layer * NB, 2 * NB)]

            def emit_xg0_chunk(c):
                """layer-0 x-gates for steps [c*C, (c+1)*C); generator with
                one (matmul + ring copy) unit per advance."""
                base_step = (c * C) % S
                for m in range(6):
                    g, j = ("r", "z", "n")[m // 2], m % 2
                    ps = pbig.tile([128, C * NB], F32, tag="big")
                    nc.tensor.matmul(ps[:],
                                     wih0[:, ts(m, 128)],
                                     xT[:, ds(c * C * NB, C * NB)],
                                     start=True, stop=True)
                    dst = xg[:, ds(base_step * GW, C * GW)]
                    dst = dst.rearrange("p (s g) -> p s g", g=GW)
                    dst = dst[:, :, ds(_ring_col(0, g, j), NB)]
                    src = ps[:].rearrange("p (s b) -> p s b", b=NB)
                    nc.scalar.activation(dst, src, AF.Copy)
                    yield

            def emit_xg1_chunk(c):
                """layer-1 x-gates for steps [c*C, (c+1)*C) from the layer-0
                h history; one (2 matmuls + ring copy) unit per advance."""
                base_step = (c * C) % S
                seg = hist[:, ds(base_step * W, C * W)]
                seg = seg.rearrange("p (s c) -> p s c", c=W)
                for m in range(6):
                    g, j = ("r", "z", "n")[m // 2], m % 2
                    ps = pbig.tile([128, C * NB], F32, tag="big")
                    for k in range(2):
                        nc.tensor.matmul(ps[:],
                                         wih1[:, ds(k * G + m * 128, 128)],
                                         seg[:, :, ds(k * NB, NB)],
                                         start=(k == 0), stop=(k == 1))
                    dst = xg[:, ds(base_step * GW, C * GW)]
                    dst = dst.rearrange("p (s g) -> p s g", g=GW)
                    dst = dst[:, :, ds(_ring_col(1, g, j), NB)]
                    src = ps[:].rearrange("p (s b) -> p s b", b=NB)
                    nc.scalar.activation(dst, src, AF.Copy)
                    yield

            def emit_regressor_chunk(rc):
                """relu(h2@W1.T+b1) @ W2.T + b2 -> relu -> out for steps
                [rc*C, (rc+1)*C) of layer 1; 3 units."""
                base_step = (rc * C) % S
                seg = hist[:, ds(base_step * W, C * W)]
                seg = seg.rearrange("p (s c) -> p s c", c=W)
                rT = work.tile([128, 2 * C * NB], DT, tag="rT")
                for mm in range(2):
                    ps = pbig.tile([128, C * NB], F32, tag="big")
                    for k in range(2):
                        nc.tensor.matmul(ps[:],
                                         w1[:, ts(mm * 2 + k, 128)],
                                         seg[:, :, ds((2 + k) * NB, NB)],
                                         start=(k == 0), stop=(k == 1))
                    nc.scalar.activation(rT[:, ts(mm, C * NB)], ps[:],
                                         AF.Relu, bias=b1[:, ds(mm, 1)])
                    yield
                po = pst2.tile([1, C * NB], F32, tag="st2")
                for k in range(2):
                    nc.tensor.matmul(po[:], w2[:, ds(k, 1)],
                                     rT[:, ts(k, C * NB)],
                                     start=(k == 0), stop=(k == 1))
                oT = work.tile([1, C * NB], F32, tag="oT")
                nc.scalar.activation(oT[:], po[:], AF.Relu, bias=b2[:, ds(0, 1)])
                nc.sync.dma_start(out_h[ds(rc, 1), :], oT[:])
                yield

            def emit_mm_group(layer, t):
                """Gate matmuls for one layer-step, two PSUM banks:
                  A [r|z|xn]: identity preload of xr|xz|xn + 8 rz MMs -> the
                    sigmoid unblocks after 9 MMs instead of 13.
                  B [hn]: 4 n MMs; the first carries start=True (a start marks
                    the whole bank pending-zero, so no preload is needed).
                Returns (psA, psB)."""
                slot = t % S
                wh = wh0 if layer == 0 else wh1
                psA = pgA.tile([128, 6 * NB], F32, tag="psA")
                psB = pgB.tile([128, 2 * NB], F32, tag="psB")
                nc.tensor.matmul(psA[:],
                                 ident[:, :],
                                 xg[:, ds(slot * GW + layer * LG, 6 * NB)],
                                 start=True, stop=False)
                mms = [(g, m, k)
                       for g in ("r", "z") for m in range(2)
                       for k in range(2)]
                for i, (g, m, k) in enumerate(mms):
                    mrow = {"r": 0, "z": 2}[g] + m
                    pcol = {"r": PS_R, "z": PS_Z}[g] + m * NB
                    nc.tensor.matmul(
                        psA[:, ds(pcol, NB)],
                        wh[:, ts(mrow * 2 + k, 128)],
                        hist_mv(layer, t - 1, k),
                        start=False, stop=(i == len(mms) - 1))
                nmm = [(m, k) for m in range(2) for k in range(2)]
                for i, (m, k) in enumerate(nmm):
                    nc.tensor.matmul(
                        psB[:, ds(m * NB, NB)],
                        wh[:, ts((4 + m) * 2 + k, 128)],
                        hist_mv(layer, t - 1, k),
                        start=(i == 0), stop=(i == len(nmm) - 1))
                return psA, psB

            def chain_gen(layer, t, psA, psB):
                """Short per-layer elementwise chain; yields between ops so
                two layers' chains interleave in emission order."""
                rz = work.tile([128, 4 * NB], BF16, tag=f"rz{layer}")
                nc.scalar.activation(rz[:], psA[:, 0:4 * NB], AF.Sigmoid)
                yield
                # z-path on GPSIMD (off the serial path, runs during tanh)
                u = work.tile([128, 2 * NB], BF16, tag=f"u{layer}")
                nc.gpsimd.tensor_scalar(u[:], rz[:, ds(2 * NB, 2 * NB)],
                                        -1.0, 1.0, ALU.mult, ALU.add)
                zh = work.tile([128, 2 * NB], F32, tag=f"zh{layer}")
                nc.gpsimd.tensor_mul(zh[:], rz[:, ds(2 * NB, 2 * NB)],
                                     hist_hcols(layer, t - 1))
                yield
                tt = work.tile([128, 2 * NB], F32, tag=f"tt{layer}")
                nc.vector.tensor_mul(tt[:], rz[:, ds(0, 2 * NB)], psB[:])
                yield
                t2 = work.tile([128, 2 * NB], F32, tag=f"t2{layer}")
                nc.vector.tensor_add(t2[:], tt[:], psA[:, ds(PS_XN, 2 * NB)])
                yield
                nn = work.tile([128, 2 * NB], BF16, tag=f"nn{layer}")
                nc.scalar.activation(nn[:], t2[:], AF.Tanh)
                yield
                nu = work.tile([128, 2 * NB], F32, tag=f"nu{layer}")
                if os.environ.get("KTAILGPS", "0") == "1":
                    nc.gpsimd.tensor_mul(nu[:], nn[:], u[:])
                    nc.gpsimd.tensor_add(hist_hcols(layer, t), nu[:], zh[:])
                else:
                    nc.vector.tensor_mul(nu[:], nn[:], u[:])
                    yield
                    nc.vector.tensor_add(hist_hcols(layer, t), nu[:], zh[:])

            def emit_round(r):
                work_items = []
                if r < T:
                    work_items.append((0, r))
                if r >= D:
                    work_items.append((1, r - D))
                gens = []
                for layer, t in work_items:
                    psA, psB = emit_mm_group(layer, t)
                    gens.append(chain_gen(layer, t, psA, psB))
                while gens:
                    gens = [g for g in gens if next(g, "done") != "done"]

            no_aux = os.environ.get("KNOAUX", "0") == "1"
            bulk_aux = os.environ.get("KSPREAD", "0") != "1"

            def drain(gen):
                for _ in gen:
                    pass

            def emit_body():
                # zero initial h slots (slot S-1 == slot(-1))
                nc.vector.memset(hist[:, ds((S - 1) * W, W)], 0.0)
                aux = []  # active aux generators, advanced 1 unit per round
                if no_aux:
                    nc.vector.memset(xg[:], 0.01)
                else:
                    drain(emit_xg0_chunk(0))
                    drain(emit_xg0_chunk(1))
                n_rounds = T + D
                for r in range(n_rounds):
                    # advance the aux queue by one unit per round, emitted
                    # ahead of the round's chain ops so the copies fill the
                    # engines' idle window at round start
                    if aux and (bulk_aux or next(aux[0], "done") == "done"):
                        if bulk_aux:
                            for g in aux:
                                drain(g)
                            aux = []
                        else:
                            aux.pop(0)
                    emit_round(r)
                    if no_aux:
                        continue
                    if r < T and (r + 1) % C == 0:
                        c = (r + 1) // C - 1  # layer-0 chunk just finished
                        if c + 2 < T // C:
                            aux.append(emit_xg0_chunk(c + 2))
                        aux.append(emit_xg1_chunk(c))
                    if r >= D and (r - D + 1) % C == 0:
                        aux.append(emit_regressor_chunk((r - D + 1) // C - 1))
                for g in aux:
                    drain(g)
                if no_aux:
                    drain(emit_regressor_chunk(0))

            if repeat == 1:
                emit_body()
            else:
                with tc.For_i(0, repeat, 1):
                    emit_body()

    nc.compile()
    return nc


_CACHE = {}


def _get_program(dt=BF16, repeat=1):
    key = (str(dt), repeat)
    if key not in _CACHE:
        _CACHE[key] = build_program(dt, repeat)
    return _CACHE[key]


def make_in_maps(inputs, np_dt=None):
    """Host-side prep: slice batch, transpose, pack biases, cast bf16."""
    import ml_dtypes
    if np_dt is None:
        np_dt = ml_dtypes.bfloat16
    x = np.asarray(inputs["x"], np.float32)
    Wih0 = np.asarray(inputs["Wih0"], np.float32)
    Whh0 = np.asarray(inputs["Whh0"], np.float32)
    bih0 = np.asarray(inputs["bih0"], np.float32)
    bhh0 = np.asarray(inputs["bhh0"], np.float32)
    Wih1 = np.asarray(inputs["Wih1"], np.float32)
    Whh1 = np.asarray(inputs["Whh1"], np.float32)
    bih1 = np.asarray(inputs["bih1"], np.float32)
    bhh1 = np.asarray(inputs["bhh1"], np.float32)
    W1 = np.asarray(inputs["W1"], np.float32)
    b1 = np.asarray(inputs["b1"], np.float32)
    W2 = np.asarray(inputs["W2"], np.float32)
    b2 = np.asarray(inputs["b2"], np.float32)

    assert not np.any(bhh0[2 * H:]) and not np.any(bhh1[2 * H:]), \
        "nonzero bhh n-gate bias not supported by this build"
    assert not np.any(bih1) and not np.any(bhh1[:2 * H]), \
        "nonzero layer-1 input bias not supported by this build"

    bias0 = np.concatenate([bih0[:2 * H] + bhh0[:2 * H], bih0[2 * H:]])
    wih0T = np.vstack([Wih0.T, bias0[None, :]]).astype(np_dt)  # [17, 768]

    shared = {
        "wh0T": Whh0.T.copy().astype(np_dt),
        "wih0T": wih0T,
        "wh1T": Whh1.T.copy().astype(np_dt),
        "wih1T": Wih1.T.copy().astype(np_dt),
        "ident": np.eye(128, dtype=np_dt),
        "w1T": W1.T.copy().astype(np_dt),
        "b1c": b1.reshape(2, 128).T.copy().astype(np.float32),
        "w2c": W2[0].reshape(2, 128).T.copy().astype(np_dt),
        "b2c": b2.reshape(1, 1).astype(np.float32),
    }
    in_maps = []
    for c in range(N_CORES):
        xc = x[c * Bc:(c + 1) * Bc]  # [8, T, 16]
        xTc = xc.transpose(2, 1, 0).reshape(I_DIM, T * Bc)  # [16, T*8]
        xTc = np.vstack([xTc, np.ones((1, T * Bc), np.float32)]).astype(np_dt)
        m = dict(shared)
        m["xT"] = xTc
        in_maps.append(m)
    return in_maps


def assemble_output(results):
    outs = []
    for c in range(N_CORES):
        r = np.asarray(results[c]["out"], np.float32)  # [32, 512]
        r = r.reshape(T // C, C, Bc).transpose(2, 0, 1).reshape(Bc, T)
        outs.append(r)
    return np.concatenate(outs, axis=0)[:, :, None]  # [64, 2048, 1]


DT_COMPUTE = BF16
NP_DT = None  # resolved to ml_dtypes.bfloat16 in make_in_maps


def kernel(**inputs):
    nc = _get_program(DT_COMPUTE, 1)
    in_maps = make_in_maps(inputs)
    res = run_bass_kernel_spmd(nc, in_maps, core_ids=list(range(N_CORES)))
    return assemble_output(res.results)
